# revision 2
# baseline (speedup 1.0000x reference)
"""KoLeo-loss kernel for Trainium2, 8 NeuronCores — fp8 DoubleRow edition.

Math: rows are L2-normalized; dist(a,b) = sqrt(2-2*a.b) for unit vectors, so
the per-row NN distance needs only the row-max of the diagonal-masked cosine
Gram. loss_i = -0.5*ln(2-2*m_i) (the torch 1e-8 is far below fp32 ulp here).

Sharding: replicate keys, shard the BxB Gram rows across 8 cores (512 rows
each). Each core returns the partial sum(-0.5*ln t)/B over its rows and both
views; the host adds the 8 partials.

Implementation notes:
  - keys are scaled to 32/||x|| and cast to fp8e4 (entries ~N(0,1) — well
    inside e4m3 range), so the Gram is 1024*cos; t = 2 - M/512.
  - fp8 y is packed 4-per-fp32-word with a strided-byte write during the
    normalize multiply (GPSIMD): word w of (v,t) holds depths t*512+b*128+w
    in byte b. PE transposes fp32 words; after transpose partition ki, word
    r, byte b = y[r, t*512+b*128+ki].
  - matmul: 4 DoubleRow fp8 matmuls per (group, view, mc): instruction b
    contracts depths {b*128+ki} paired over t (i-axis stride = one t-tile,
    16B-aligned as the ISA requires; m/n axes stride 4B — walrus-validated).
  - diagonal mask: one N=128 bf16 matmul accumulating -8192*gsel[g]*I into
    the diagonal band of the PSUM block (gsel is a host-fed one-hot of the
    core's own column-group).
  - DVE reduce_max per block -> finale -0.5*ln(2-M/512), partition-sum via
    ones-matmul, scalar out.
"""

import sys
from contextlib import ExitStack

import numpy as np

sys.path.insert(0, "/opt/trn_rl_repo")

import concourse.mybir as mybir
import concourse.tile as tile
from concourse import bacc, bass_utils

F32 = mybir.dt.float32
BF16 = mybir.dt.bfloat16
F8 = mybir.dt.float8e4
AF = mybir.ActivationFunctionType
DR = mybir.MatmulPerfMode.DoubleRow

B, V, D = 4096, 2, 1024
NCORES = 8
MB = B // NCORES          # 512 own rows per core
NCHUNK = B // 128         # 32 key chunks
NQ = MB // 128            # 4 own chunks
NG = 8                    # column groups of 512 keys
T = 2                     # transpose tiles per view (512 depths each)
EPS = 1e-8
MASKV = -8192.0           # diag delta; 1024 + MASKV < -1024 <= any true max


def _process_chunk(nc, pools, x_src, dstT, col0):
    """Load one [128, V, D] fp32 chunk, normalize+scale to fp8 (32/||x||),
    pack into fp32 words (byte b = depth subtile b), transpose into
    dstT[:, v, t, col0:col0+128] word layout."""
    xpool, ypool, sqpool, sspool, trp, identF = pools
    xt = xpool.tile([128, V, D], F32, tag="xraw", name="xraw")
    nc.sync.dma_start(xt[:], x_src)

    ss = sspool.tile([128, V], F32, tag="ss", name="ss")
    sq = sqpool.tile([128, D], BF16, tag="sq", name="sq")
    for v in range(V):
        nc.scalar.activation(sq[:], xt[:, v, :], AF.Square, accum_out=ss[:, v : v + 1])
    rec = sspool.tile([128, V], F32, tag="rec", name="rec")
    nc.vector.tensor_scalar_add(rec[:], ss[:], EPS)
    nc.vector.reciprocal(rec[:], rec[:])
    # rs = sqrt(1024/(ss+eps)) = 32/||x||  (Sqrt shares a table set w/ Square)
    rs = sspool.tile([128, V], F32, tag="rs", name="rs")
    nc.scalar.activation(rs[:], rec[:], AF.Sqrt, scale=1024.0)
    # same-engine staging: the pack op then carries a single cross-engine wait
    rsl = sspool.tile([128, V], F32, tag="rsl", name="rsl")
    nc.gpsimd.tensor_copy(rsl[:], rs[:])

    ypk = ypool.tile([128, V, T, 128], F32, tag="ypk", name="ypk")
    yp8 = ypk.bitcast(F8)  # [128, V, T, 512]
    for v in range(V):
        nc.gpsimd.tensor_scalar_mul(
            yp8[:, v].rearrange("p t (w b) -> p t b w", b=4),
            xt[:, v, :].rearrange("p (t b w) -> p t b w", t=T, b=4),
            rsl[:, v : v + 1])

    for v in range(V):
        for t in range(T):
            tp = trp.tile([128, 128], F32, tag="tp", name="tp")
            nc.tensor.transpose(tp[:], ypk[:, v, t], identF[:])
            nc.vector.tensor_copy(dstT[:, v, t, col0 : col0 + 128], tp[:])


def build():
    nc = bacc.Bacc("TRN2", debug=False)
    x_d = nc.dram_tensor("x", [B, V, D], F32, kind="ExternalInput").ap()
    xq_d = nc.dram_tensor("xq", [MB, V, D], F32, kind="ExternalInput").ap()
    gs_d = nc.dram_tensor("gsel", [128, NG], F32, kind="ExternalInput").ap()
    out_d = nc.dram_tensor("out", [1, 1], F32, kind="ExternalOutput").ap()

    with ExitStack() as ctx:
        tc = ctx.enter_context(tile.TileContext(nc))
        const = ctx.enter_context(tc.tile_pool(name="const", bufs=1))
        xpool = ctx.enter_context(tc.tile_pool(name="xpool", bufs=4))
        ypool = ctx.enter_context(tc.tile_pool(name="ypool", bufs=3))
        sqpool = ctx.enter_context(tc.tile_pool(name="sqpool", bufs=2))
        sspool = ctx.enter_context(tc.tile_pool(name="sspool", bufs=3))
        accp = ctx.enter_context(tc.tile_pool(name="accp", bufs=3, space="PSUM"))
        trp = ctx.enter_context(tc.tile_pool(name="trp", bufs=3, space="PSUM"))
        smallp = ctx.enter_context(tc.tile_pool(name="smallp", bufs=2, space="PSUM"))

        # ---- constants ----
        identF = const.tile([128, 128], F32, name="identF")
        nc.gpsimd.memset(identF[:], 0.0)
        nc.gpsimd.affine_select(
            out=identF[:], in_=identF[:], compare_op=mybir.AluOpType.not_equal,
            fill=1.0, base=0, pattern=[[-1, 128]], channel_multiplier=1)

        identB = const.tile([128, 128], BF16, name="identB")
        nc.gpsimd.memset(identB[:], 0.0)
        nc.gpsimd.affine_select(
            out=identB[:], in_=identB[:], compare_op=mybir.AluOpType.not_equal,
            fill=1.0, base=0, pattern=[[-1, 128]], channel_multiplier=1)

        negI = const.tile([128, 128], BF16, name="negI")
        nc.gpsimd.memset(negI[:], 0.0)
        nc.gpsimd.affine_select(
            out=negI[:], in_=negI[:], compare_op=mybir.AluOpType.not_equal,
            fill=MASKV, base=0, pattern=[[-1, 128]], channel_multiplier=1)

        ones = const.tile([128, 1], F32, name="ones")
        nc.vector.memset(ones[:], 1.0)
        epsb = const.tile([128, 1], F32, name="epsb")
        nc.gpsimd.memset(epsb[:], EPS)

        # gsel arrives host-replicated [128, NG]; gselI[:, g] = MASKV*I*gsel[g]
        gsbc = const.tile([128, NG], F32, name="gsbc")
        nc.sync.dma_start(gsbc[:], gs_d)
        gselI = const.tile([128, NG, 128], BF16, name="gselI")
        for g in range(NG):
            nc.gpsimd.tensor_scalar_mul(gselI[:, g, :], negI[:], gsbc[:, g : g + 1])

        # HAM warmup: keep PE busy early so the clock gate opens before the
        # first real matmuls; also advances PE's observed gpsimd clock.
        wrm = trp.tile([128, 128], F32, tag="tp", name="wrm")
        for _ in range(20):
            nc.tensor.transpose(wrm[:], identF[:], identF[:])

        # ---- persistent transposed buffers (fp32 words of packed fp8) ----
        QT = const.tile([128, V, T, MB], F32, name="QT")
        YTg = [const.tile([128, V, T, 512], F32, name=f"YT{g}") for g in range(NG)]
        mxs = const.tile([128, NG, V * NQ], F32, name="mxs")

        pools = (xpool, ypool, sqpool, sspool, trp, identF)

        # ---- own rows -> QT ----
        for qc in range(NQ):
            _process_chunk(nc, pools, xq_d[128 * qc : 128 * (qc + 1)], QT, 128 * qc)
        Q8r = QT.bitcast(F8)[:].rearrange("p v t (m b) -> p v b t m", b=4)

        # ---- stream groups ----
        for g in range(NG):
            for c4 in range(4):
                gc = 4 * g + c4
                _process_chunk(
                    nc, pools, x_d[128 * gc : 128 * (gc + 1)], YTg[g], 128 * c4)
            Y8r = YTg[g].bitcast(F8)[:].rearrange("p v t (k b) -> p v b t k", b=4)
            for v in range(V):
                for mc in range(NQ):
                    acc = accp.tile([128, 512], F32, tag="acc", name="acc")
                    for b in range(4):
                        nc.tensor.matmul(
                            acc[:],
                            Q8r[:, v, b, :, 128 * mc : 128 * (mc + 1)],
                            Y8r[:, v, b, :, :],
                            start=(b == 0), stop=False, perf_mode=DR)
                    nc.tensor.matmul(
                        acc[:, 128 * mc : 128 * (mc + 1)],
                        gselI[:, g], identB[:],
                        start=False, stop=True, skip_group_check=True)
                    nc.vector.reduce_max(
                        mxs[:, g, v * NQ + mc : v * NQ + mc + 1], acc[:],
                        axis=mybir.AxisListType.X)

        # ---- finale: loss partial = -0.5/B * sum(ln(2 - M/512)) ----
        fm = const.tile([128, V * NQ], F32, name="fm")
        nc.vector.reduce_max(
            fm[:], mxs.rearrange("p g c -> p c g"), axis=mybir.AxisListType.X)
        tt = const.tile([128, V * NQ], F32, name="tt")
        nc.vector.tensor_scalar(
            tt[:], fm[:], -1.0 / 512.0, 2.0, mybir.AluOpType.mult,
            mybir.AluOpType.add)
        lg = const.tile([128, V * NQ], F32, name="lg")
        nc.scalar.activation(lg[:], tt[:], AF.Ln, bias=epsb[:])
        ps2 = smallp.tile([1, V * NQ], F32, tag="sps", name="ps2")
        nc.tensor.matmul(ps2[:], ones[:], lg[:], start=True, stop=True)
        tot = const.tile([1, 1], F32, name="tot")
        nc.vector.reduce_sum(tot[:], ps2[:], axis=mybir.AxisListType.X)
        tots = const.tile([1, 1], F32, name="tots")
        nc.vector.tensor_scalar_mul(tots[:], tot[:], -0.5 / B)
        nc.sync.dma_start(out_d, tots[:])

    nc.compile()
    return nc


_CACHED = {}


def _run(x, trace=False):
    x = np.ascontiguousarray(np.asarray(x, dtype=np.float32))
    assert x.shape == (B, V, D), x.shape
    if "nc" not in _CACHED:
        _CACHED["nc"] = build()
    nc = _CACHED["nc"]
    in_maps = []
    for r in range(NCORES):
        gsel = np.zeros((1, NG), np.float32)
        gsel[0, r] = 1.0
        in_maps.append({
            "x": x,
            "xq": np.ascontiguousarray(x[MB * r : MB * (r + 1)]),
            "gsel": np.broadcast_to(gsel, (128, NG)).copy(),
        })
    res = bass_utils.run_bass_kernel_spmd(
        nc, in_maps, core_ids=list(range(NCORES)), trace=trace)
    partials = [np.float32(res.results[r]["out"][0, 0]) for r in range(NCORES)]
    total = np.float32(np.sum(np.array(partials, dtype=np.float32)))
    return total, res


def kernel(student_global_cls_tokens):
    total, _ = _run(student_global_cls_tokens, trace=False)
    return np.asarray(total, dtype=np.float32)


# revision 4
# speedup vs baseline: 7.2008x; 7.2008x over previous
"""KoLeo-loss kernel for Trainium2, 8 NeuronCores — fp8 DoubleRow edition.

Math: rows are L2-normalized; dist(a,b) = sqrt(2-2*a.b) for unit vectors, so
the per-row NN distance needs only the row-max of the diagonal-masked cosine
Gram. loss_i = -0.5*ln(2-2*m_i) (the torch 1e-8 is far below fp32 ulp here).

Sharding: replicate keys, shard the BxB Gram rows across 8 cores (512 rows
each). Each core returns the partial sum(-0.5*ln t)/B over its rows and both
views; the host adds the 8 partials.

Implementation notes:
  - keys are scaled to 32/||x|| and cast to fp8e4 (entries ~N(0,1) — well
    inside e4m3 range), so the Gram is 1024*cos; t = 2 - M/512.
  - fp8 y is packed 4-per-fp32-word with a strided-byte write during the
    normalize multiply (GPSIMD): word w of (v,t) holds depths t*512+b*128+w
    in byte b. PE transposes fp32 words; after transpose partition ki, word
    r, byte b = y[r, t*512+b*128+ki].
  - matmul: 4 DoubleRow fp8 matmuls per (group, view, mc): instruction b
    contracts depths {b*128+ki} paired over t (i-axis stride = one t-tile,
    16B-aligned as the ISA requires; m/n axes stride 4B — walrus-validated).
  - diagonal mask: one N=128 bf16 matmul accumulating -8192*gsel[g]*I into
    the diagonal band of the PSUM block (gsel is a host-fed one-hot of the
    core's own column-group).
  - DVE reduce_max per block -> finale -0.5*ln(2-M/512), partition-sum via
    ones-matmul, scalar out.
"""

import sys
from contextlib import ExitStack

import numpy as np

sys.path.insert(0, "/opt/trn_rl_repo")

import concourse.mybir as mybir
import concourse.tile as tile
from concourse import bacc, bass_utils

F32 = mybir.dt.float32
BF16 = mybir.dt.bfloat16
F8 = mybir.dt.float8e4
AF = mybir.ActivationFunctionType
DR = mybir.MatmulPerfMode.DoubleRow

B, V, D = 4096, 2, 1024
NCORES = 8
MB = B // NCORES          # 512 own rows per core
NCHUNK = B // 128         # 32 key chunks
NQ = MB // 128            # 4 own chunks
NG = 8                    # column groups of 512 keys
T = 2                     # transpose tiles per view (512 depths each)
EPS = 1e-8
MASKV = -8192.0           # diag delta; 1024 + MASKV < -1024 <= any true max


def _process_chunk(nc, pools, x_src, dstT, col0):
    """Load one [128, V, D] fp32 chunk, normalize+scale to fp8 (32/||x||),
    pack into fp32 words (byte b = depth subtile b), transpose into
    dstT[:, v, t, col0:col0+128] word layout."""
    xpool, ypool, sqpool, sspool, trp, identF = pools
    xt = xpool.tile([128, V, D], F32, tag="xraw", name="xraw")
    nc.sync.dma_start(xt[:], x_src)

    ss = sspool.tile([128, V], F32, tag="ss", name="ss")
    sq = sqpool.tile([128, D], BF16, tag="sq", name="sq")
    for v in range(V):
        nc.scalar.activation(sq[:], xt[:, v, :], AF.Square, accum_out=ss[:, v : v + 1])
    rec = sspool.tile([128, V], F32, tag="rec", name="rec")
    nc.vector.tensor_scalar_add(rec[:], ss[:], EPS)
    nc.vector.reciprocal(rec[:], rec[:])
    # rs = sqrt(1024/(ss+eps)) = 32/||x||  (Sqrt shares a table set w/ Square)
    rs = sspool.tile([128, V], F32, tag="rs", name="rs")
    nc.scalar.activation(rs[:], rec[:], AF.Sqrt, scale=1024.0)
    # DVE-local staging so the DVE cast carries a single cross-engine wait
    rsv = sspool.tile([128, V], F32, tag="rsv", name="rsv")
    nc.vector.tensor_copy(rsv[:], rs[:])

    # contiguous fp8 cast y = rs*x, one view per engine (ACT / DVE).
    # Packed word w of (v,t) holds depths t*512+4w+b in byte b; the matmul's
    # t-paired DoubleRow APs only need this layout to agree between QT and YT.
    ypk = ypool.tile([128, V, T, 128], F32, tag="ypk", name="ypk")
    yp8 = ypk.bitcast(F8)  # [128, V, 2*512]
    nc.scalar.activation(
        yp8[:, 0].rearrange("p t k -> p (t k)"), xt[:, 0, :], AF.Copy,
        scale=rs[:, 0:1])
    nc.vector.tensor_scalar_mul(
        yp8[:, 1].rearrange("p t k -> p (t k)"), xt[:, 1, :], rsv[:, 1:2])

    for v in range(V):
        for t in range(T):
            tp = trp.tile([128, 128], F32, tag="tp", name="tp")
            nc.tensor.transpose(tp[:], ypk[:, v, t], identF[:])
            nc.vector.tensor_copy(dstT[:, v, t, col0 : col0 + 128], tp[:])


def build():
    nc = bacc.Bacc("TRN2", debug=False)
    x_d = nc.dram_tensor("x", [B, V, D], F32, kind="ExternalInput").ap()
    xq_d = nc.dram_tensor("xq", [MB, V, D], F32, kind="ExternalInput").ap()
    gs_d = nc.dram_tensor("gsel", [128, NG], F32, kind="ExternalInput").ap()
    out_d = nc.dram_tensor("out", [1, 1], F32, kind="ExternalOutput").ap()

    with ExitStack() as ctx:
        tc = ctx.enter_context(tile.TileContext(nc))
        const = ctx.enter_context(tc.tile_pool(name="const", bufs=1))
        xpool = ctx.enter_context(tc.tile_pool(name="xpool", bufs=4))
        ypool = ctx.enter_context(tc.tile_pool(name="ypool", bufs=3))
        sqpool = ctx.enter_context(tc.tile_pool(name="sqpool", bufs=2))
        sspool = ctx.enter_context(tc.tile_pool(name="sspool", bufs=3))
        accp = ctx.enter_context(tc.tile_pool(name="accp", bufs=3, space="PSUM"))
        trp = ctx.enter_context(tc.tile_pool(name="trp", bufs=3, space="PSUM"))
        smallp = ctx.enter_context(tc.tile_pool(name="smallp", bufs=2, space="PSUM"))

        # ---- constants ----
        identF = const.tile([128, 128], F32, name="identF")
        nc.gpsimd.memset(identF[:], 0.0)
        nc.gpsimd.affine_select(
            out=identF[:], in_=identF[:], compare_op=mybir.AluOpType.not_equal,
            fill=1.0, base=0, pattern=[[-1, 128]], channel_multiplier=1)

        identB = const.tile([128, 128], BF16, name="identB")
        nc.gpsimd.memset(identB[:], 0.0)
        nc.gpsimd.affine_select(
            out=identB[:], in_=identB[:], compare_op=mybir.AluOpType.not_equal,
            fill=1.0, base=0, pattern=[[-1, 128]], channel_multiplier=1)

        negI = const.tile([128, 128], BF16, name="negI")
        nc.gpsimd.memset(negI[:], 0.0)
        nc.gpsimd.affine_select(
            out=negI[:], in_=negI[:], compare_op=mybir.AluOpType.not_equal,
            fill=MASKV, base=0, pattern=[[-1, 128]], channel_multiplier=1)

        ones = const.tile([128, 1], F32, name="ones")
        nc.vector.memset(ones[:], 1.0)
        epsb = const.tile([128, 1], F32, name="epsb")
        nc.gpsimd.memset(epsb[:], EPS)

        # gsel arrives host-replicated [128, NG]; gselI[:, g] = MASKV*I*gsel[g]
        gsbc = const.tile([128, NG], F32, name="gsbc")
        nc.sync.dma_start(gsbc[:], gs_d)
        gselI = const.tile([128, NG, 128], BF16, name="gselI")
        for g in range(NG):
            nc.gpsimd.tensor_scalar_mul(gselI[:, g, :], negI[:], gsbc[:, g : g + 1])

        # HAM warmup: keep PE busy early so the clock gate opens before the
        # first real matmuls; also advances PE's observed gpsimd clock.
        wrm = trp.tile([128, 128], F32, tag="tp", name="wrm")
        for _ in range(20):
            nc.tensor.transpose(wrm[:], identF[:], identF[:])

        # ---- persistent transposed buffers (fp32 words of packed fp8) ----
        QT = const.tile([128, V, T, MB], F32, name="QT")
        YTg = [const.tile([128, V, T, 512], F32, name=f"YT{g}") for g in range(NG)]
        mxs = const.tile([128, NG, V * NQ], F32, name="mxs")

        pools = (xpool, ypool, sqpool, sspool, trp, identF)

        # ---- own rows -> QT ----
        for qc in range(NQ):
            _process_chunk(nc, pools, xq_d[128 * qc : 128 * (qc + 1)], QT, 128 * qc)
        Q8r = QT.bitcast(F8)[:].rearrange("p v t (m b) -> p v b t m", b=4)

        # ---- stream groups ----
        for g in range(NG):
            for c4 in range(4):
                gc = 4 * g + c4
                _process_chunk(
                    nc, pools, x_d[128 * gc : 128 * (gc + 1)], YTg[g], 128 * c4)
            Y8r = YTg[g].bitcast(F8)[:].rearrange("p v t (k b) -> p v b t k", b=4)
            for v in range(V):
                for mc in range(NQ):
                    acc = accp.tile([128, 512], F32, tag="acc", name="acc")
                    for b in range(4):
                        nc.tensor.matmul(
                            acc[:],
                            Q8r[:, v, b, :, 128 * mc : 128 * (mc + 1)],
                            Y8r[:, v, b, :, :],
                            start=(b == 0), stop=False, perf_mode=DR)
                    nc.tensor.matmul(
                        acc[:, 128 * mc : 128 * (mc + 1)],
                        gselI[:, g], identB[:],
                        start=False, stop=True, skip_group_check=True)
                    nc.vector.reduce_max(
                        mxs[:, g, v * NQ + mc : v * NQ + mc + 1], acc[:],
                        axis=mybir.AxisListType.X)

        # ---- finale: loss partial = -0.5/B * sum(ln(2 - M/512)) ----
        fm = const.tile([128, V * NQ], F32, name="fm")
        nc.vector.reduce_max(
            fm[:], mxs.rearrange("p g c -> p c g"), axis=mybir.AxisListType.X)
        tt = const.tile([128, V * NQ], F32, name="tt")
        nc.vector.tensor_scalar(
            tt[:], fm[:], -1.0 / 512.0, 2.0, mybir.AluOpType.mult,
            mybir.AluOpType.add)
        lg = const.tile([128, V * NQ], F32, name="lg")
        nc.scalar.activation(lg[:], tt[:], AF.Ln, bias=epsb[:])
        ps2 = smallp.tile([1, V * NQ], F32, tag="sps", name="ps2")
        nc.tensor.matmul(ps2[:], ones[:], lg[:], start=True, stop=True)
        tot = const.tile([1, 1], F32, name="tot")
        nc.vector.reduce_sum(tot[:], ps2[:], axis=mybir.AxisListType.X)
        tots = const.tile([1, 1], F32, name="tots")
        nc.vector.tensor_scalar_mul(tots[:], tot[:], -0.5 / B)
        nc.sync.dma_start(out_d, tots[:])

    nc.compile()
    return nc


_CACHED = {}


def _run(x, trace=False):
    x = np.ascontiguousarray(np.asarray(x, dtype=np.float32))
    assert x.shape == (B, V, D), x.shape
    if "nc" not in _CACHED:
        _CACHED["nc"] = build()
    nc = _CACHED["nc"]
    in_maps = []
    for r in range(NCORES):
        gsel = np.zeros((1, NG), np.float32)
        gsel[0, r] = 1.0
        in_maps.append({
            "x": x,
            "xq": np.ascontiguousarray(x[MB * r : MB * (r + 1)]),
            "gsel": np.broadcast_to(gsel, (128, NG)).copy(),
        })
    res = bass_utils.run_bass_kernel_spmd(
        nc, in_maps, core_ids=list(range(NCORES)), trace=trace)
    partials = [np.float32(res.results[r]["out"][0, 0]) for r in range(NCORES)]
    total = np.float32(np.sum(np.array(partials, dtype=np.float32)))
    return total, res


def kernel(student_global_cls_tokens):
    total, _ = _run(student_global_cls_tokens, trace=False)
    return np.asarray(total, dtype=np.float32)
